# revision 9
# baseline (speedup 1.0000x reference)
"""ARMA GNN conv on 8 TRN2 NeuronCores — block-dense SpMM formulation.

Math (reference):
    out0 = x @ W.T + b
    out1 = out0 + w0 * (A @ out0)
    q    = A @ out1
    out2 = out1 + w1 * (A @ q)
where A is the symmetric-normalized adjacency: A[i,j] = sum over edges
(row=i, col=j) of dis[i]*dis[j], dis = rsqrt(bincount(row)).

Distribution: destination nodes sharded 1280/core (N padded 10000->10240).
Each hop: all-gather the bf16 feature table (10240x256) to HBM, stream it
into SBUF, then per 128-dst tile accumulate 80 block matmuls in PSUM:
    PSUM[dst,ch] += AT_block[src,dst].T @ h_block[src,ch]
with AT blocks (128x128 bf16, hop-weight folded in) prebuilt on the host.
No per-edge gathers on device at all.
"""

import sys
import numpy as np

sys.path.insert(0, "/opt/trn_rl_repo")

import ml_dtypes  # noqa: E402

N = 10000
E = 640000
IN_CH = 512
OUT_CH = 256
CORES = 8
NPAD = 10240
SHARD = NPAD // CORES      # 1280
TPC = SHARD // 128         # 10 dst tiles per core
NT = NPAD // 128           # 80 node tiles global
ICT = IN_CH // 128         # 4 contraction chunks for the linear

BF16 = ml_dtypes.bfloat16

_CACHE = {}
LAST_RESULT = None
RUN_KWARGS = {}


def _preprocess(x, edge_index, lin_w, lin_b, weights):
    row = np.asarray(edge_index[0], dtype=np.int64)
    col = np.asarray(edge_index[1], dtype=np.int64)
    deg = np.bincount(row, minlength=N).astype(np.float64)
    dis = np.where(deg > 0, 1.0 / np.sqrt(deg), 0.0)
    norm = (dis[row] * dis[col]).astype(np.float32)

    # AT[t, p, s, d]: dst tile t, src partition p, src tile s, dst lane d.
    # lhsT layout for matmul: out[d, ch] += AT[t, :, s, :].T @ h[s*128+p, ch].
    at = np.zeros((NT, 128, NT, 128), np.float32)
    np.add.at(at, (row >> 7, col & 127, col >> 7, row & 127), norm)

    w0, w1 = float(weights[0]), float(weights[1])
    ab = np.empty((3, NT, 128, NT * 128), BF16)
    flat = at.reshape(NT, 128, NT * 128)
    ab[0] = (w0 * flat).astype(BF16)
    ab[1] = flat.astype(BF16)
    ab[2] = (w1 * flat).astype(BF16)
    # per-core: [3, TPC, 128, NT*128]
    ab = np.ascontiguousarray(
        ab.reshape(3, CORES, TPC, 128, NT * 128).transpose(1, 0, 2, 3, 4)
    )

    xpad = np.zeros((NPAD, IN_CH), np.float32)
    xpad[:N] = x
    xt = np.ascontiguousarray(
        xpad.reshape(CORES, SHARD, ICT, 128).transpose(0, 2, 3, 1)
    )  # [c, k, 128, 1280]

    wt = np.ascontiguousarray(
        np.asarray(lin_w, np.float32).T.reshape(ICT, 128, OUT_CH)
    )
    b_rep = np.ascontiguousarray(
        np.broadcast_to(np.asarray(lin_b, np.float32), (128, OUT_CH))
    )
    return ab, xt, wt, b_rep


def _build():
    import concourse.bass as bass
    import concourse.bacc as bacc
    import concourse.mybir as mybir

    f32 = mybir.dt.float32
    bf16 = mybir.dt.bfloat16

    nc = bacc.Bacc("TRN2")

    xt_d = nc.declare_dram_parameter("xt", [ICT, 128, SHARD], f32, isOutput=False)
    wt_d = nc.declare_dram_parameter("wt", [ICT, 128, OUT_CH], f32, isOutput=False)
    b_d = nc.declare_dram_parameter("b_rep", [128, OUT_CH], f32, isOutput=False)
    ab_d = nc.declare_dram_parameter("ab", [3, TPC, 128, NT * 128], bf16, isOutput=False)
    out_d = nc.declare_dram_parameter("out", [128, TPC * OUT_CH], f32, isOutput=True)

    ag_in = nc.dram_tensor("ag_in", [SHARD, OUT_CH], bf16)
    tbl = nc.dram_tensor("tbl", [NPAD, OUT_CH], bf16)

    with (
        nc.Block() as block,
        nc.semaphore("in_sem") as in_sem,
        nc.semaphore("ab_sem0") as ab_sem0,
        nc.semaphore("ab_sem1") as ab_sem1,
        nc.semaphore("tb_sem") as tb_sem,
        nc.semaphore("ag_sem") as ag_sem,
        nc.semaphore("cc_sem") as cc_sem,
        nc.semaphore("mm_sem") as mm_sem,
        nc.semaphore("v_sem") as v_sem,
        nc.semaphore("out_sem") as out_sem,
        nc.sbuf_tensor("x_sb", [128, ICT, SHARD], f32) as x_sb,
        nc.sbuf_tensor("wt_sb", [128, ICT, OUT_CH], f32) as wt_sb,
        nc.sbuf_tensor("b_sb", [128, OUT_CH], f32) as b_sb,
        nc.sbuf_tensor("ab_sb", [128, 2, NT * 128], bf16) as ab_sb,
        nc.sbuf_tensor("tb_sb", [128, NT, OUT_CH], bf16) as tb_sb,
        nc.sbuf_tensor("oc_sb", [128, TPC, OUT_CH], f32) as oc_sb,
        nc.sbuf_tensor("ag_sb", [128, TPC, OUT_CH], bf16) as ag_sb,
        nc.psum_tensor("p0", [128, OUT_CH], f32) as p0,
        nc.psum_tensor("p1", [128, OUT_CH], f32) as p1,
    ):
        psum = [p0, p1]
        ab_sems = [ab_sem0, ab_sem1]
        N_IN_DMAS = 2 * ICT + 1

        @block.sync
        def _(sync: "bass.BassEngine"):
            for k in range(ICT):
                sync.dma_start(out=x_sb[:, k, :], in_=xt_d[k]).then_inc(in_sem, 16)
                sync.dma_start(out=wt_sb[:, k, :], in_=wt_d[k]).then_inc(in_sem, 16)
            sync.dma_start(out=b_sb[:, :], in_=b_d[:, :]).then_inc(in_sem, 16)
            # A^T block loads, double buffered across the 30 (hop, dst-tile) pairs
            for s in range(3):
                for t in range(TPC):
                    gt = s * TPC + t
                    if gt >= 2:
                        # matmuls of tile gt-2 (same buffer) must be done
                        sync.wait_ge(mm_sem, TPC + gt - 1)
                    sync.dma_start(
                        out=ab_sb[:, gt % 2, :],
                        in_=ab_d[s, t],
                    ).then_inc(ab_sems[gt % 2], 16)
            sync.wait_ge(v_sem, 4 * TPC)
            sync.dma_start(
                out=out_d[:, :],
                in_=oc_sb.ap().rearrange("p t c -> p (t c)"),
            ).then_inc(out_sem, 16)
            sync.wait_ge(out_sem, 16)

        @block.gpsimd
        def _(gpsimd: bass.BassGpSimd):
            gpsimd.wait_ge(in_sem, N_IN_DMAS * 16)
            for s in range(3):
                # shard features for this hop are ready after the 10 casts
                gpsimd.wait_ge(v_sem, (s + 1) * TPC)
                gpsimd.dma_start(
                    out=ag_in.ap().rearrange("(t p) c -> p t c", p=128),
                    in_=ag_sb[:, :, :],
                ).then_inc(ag_sem, 16)
                gpsimd.wait_ge(ag_sem, (s + 1) * 16)
                gpsimd.collective_compute(
                    "AllGather",
                    mybir.AluOpType.bypass,
                    replica_groups=[list(range(CORES))],
                    ins=[ag_in.ap().opt()],
                    outs=[tbl.ap().opt()],
                ).then_inc(cc_sem)
                gpsimd.wait_ge(cc_sem, s + 1)
                # stream the gathered table into SBUF (src-tile major)
                gpsimd.dma_start(
                    out=tb_sb[:, :, :],
                    in_=tbl.ap().rearrange("(s p) c -> p s c", p=128),
                ).then_inc(tb_sem, 16)

        @block.tensor
        def _(tensor: "bass.BassTensorEngine"):
            tensor.wait_ge(in_sem, N_IN_DMAS * 16)
            # linear: out0[n, oc] = sum_ic x[n, ic] W[oc, ic]
            for t in range(TPC):
                if t >= 2:
                    tensor.wait_ge(v_sem, t - 1)
                for k in range(ICT):
                    mm = tensor.matmul(
                        psum[t % 2][:, :],
                        x_sb[:, k, t * 128:(t + 1) * 128],
                        wt_sb[:, k, :],
                        start=(k == 0),
                        stop=(k == ICT - 1),
                    )
                    if k == ICT - 1:
                        mm.then_inc(mm_sem, 1)
            # 3 hops of block-dense SpMM
            for s in range(3):
                for t in range(TPC):
                    gt = s * TPC + t
                    q = TPC + gt
                    tensor.wait_ge(v_sem, q - 1)
                    tensor.wait_ge(tb_sem, (s + 1) * 16)
                    tensor.wait_ge(ab_sems[gt % 2], (gt // 2 + 1) * 16)
                    for j in range(NT):
                        mm = tensor.matmul(
                            psum[q % 2][:, :],
                            ab_sb[:, gt % 2, j * 128:(j + 1) * 128],
                            tb_sb[:, j, :],
                            start=(j == 0),
                            stop=(j == NT - 1),
                        )
                        if j == NT - 1:
                            mm.then_inc(mm_sem, 1)

        @block.vector
        def _(vector: "bass.BassVectorEngine"):
            vector.wait_ge(in_sem, N_IN_DMAS * 16)
            # linear epilogue: oc = psum + b (f32) and ag = bf16(psum + b)
            for t in range(TPC):
                vector.wait_ge(mm_sem, t + 1)
                vector.tensor_tensor(
                    out=oc_sb[:, t, :], in0=psum[t % 2][:, :], in1=b_sb[:, :],
                    op=mybir.AluOpType.add,
                )
                vector.tensor_tensor(
                    out=ag_sb[:, t, :], in0=psum[t % 2][:, :], in1=b_sb[:, :],
                    op=mybir.AluOpType.add,
                ).then_inc(v_sem, 1)
            vector.drain()  # oc writes commit before hop-0 reads them
            for s in range(3):
                for t in range(TPC):
                    gt = s * TPC + t
                    q = TPC + gt
                    vector.wait_ge(mm_sem, q + 1)
                    if s == 0:
                        # psum = w0*(A@out0); out1 = out0 + psum
                        vector.tensor_tensor(
                            out=ag_sb[:, t, :], in0=oc_sb[:, t, :],
                            in1=psum[q % 2][:, :], op=mybir.AluOpType.add,
                        )
                        vector.drain()
                        vector.tensor_tensor(
                            out=oc_sb[:, t, :], in0=oc_sb[:, t, :],
                            in1=psum[q % 2][:, :], op=mybir.AluOpType.add,
                        ).then_inc(v_sem, 1)
                    elif s == 1:
                        # psum = q = A@out1: stage for the next hop only
                        vector.tensor_copy(
                            out=ag_sb[:, t, :], in_=psum[q % 2][:, :],
                        ).then_inc(v_sem, 1)
                    else:
                        if t == 0:
                            vector.drain()  # hop-0 oc writes commit
                        # psum = w1*(A@q); out2 = out1 + psum
                        vector.tensor_tensor(
                            out=oc_sb[:, t, :], in0=oc_sb[:, t, :],
                            in1=psum[q % 2][:, :], op=mybir.AluOpType.add,
                        ).then_inc(v_sem, 1)

    nc.compile()
    return nc


def kernel(x, edge_index, lin_w, lin_b, weights):
    global LAST_RESULT
    from concourse.bass_utils import run_bass_kernel_spmd

    x = np.asarray(x, np.float32)
    lin_w = np.asarray(lin_w, np.float32)
    lin_b = np.asarray(lin_b, np.float32)
    weights = np.asarray(weights, np.float32)

    ab, xt, wt, b_rep = _preprocess(x, edge_index, lin_w, lin_b, weights)

    key = "block_dense"
    if key not in _CACHE:
        _CACHE[key] = _build()
    nc = _CACHE[key]

    in_maps = [
        {
            "xt": xt[c],
            "wt": wt,
            "b_rep": b_rep,
            "ab": ab[c],
        }
        for c in range(CORES)
    ]
    res = run_bass_kernel_spmd(nc, in_maps, core_ids=list(range(CORES)), **RUN_KWARGS)
    LAST_RESULT = res

    shards = []
    for c in range(CORES):
        o = np.asarray(res.results[c]["out"], np.float32)
        shards.append(o.reshape(128, TPC, OUT_CH).transpose(1, 0, 2).reshape(SHARD, OUT_CH))
    full = np.concatenate(shards, axis=0)
    return full[:N]
